# revision 1
# baseline (speedup 1.0000x reference)
"""Trainium2 Bass kernel for the DriftingPolicy loss (8-core SPMD).

Math (value-equivalent to the reference):
  loss = mean(V_total^2) over [N, D], where for each temperature T in
  {0.05, 0.1, 0.2} (written as T = 0.2 / t_hat, t_hat in {4, 2, 1}):
    d[i, n]   = dist(x_i, y_n) over cols n = [y_neg | y_pos], diag of the
                neg block poisoned to a huge value (reference adds 1e6).
    K = exp(-d / (T * mean(d_pos)));  c_n = col sums;  s_i = row sums
    K' = K / sqrt(c_n)
    V += (rn_i/s_i) * (K'_pos @ y_pos) - (rp_i/s_i) * (K'_neg @ y_neg)
       where rn_i = sum_neg K', rp_i = sum_pos K', s_i = sum_all K' * sqrt(c)

Sharding: rows of x strided across 8 cores (core c gets x[c::8]) so the
neg-block diagonal lands on a core-independent local pattern; y_pos/y_neg
replicated. Two all-reduce rounds: sum(d_pos) scalar, and per-temperature
column sums. Everything is computed in a column-major ("K transposed",
[n-partition, i-free]) layout so the second matmul needs no on-chip
transposes; host pre-transposes/casts the small inputs.
"""

import sys

if "/opt/trn_rl_repo" not in sys.path:
    sys.path.insert(0, "/opt/trn_rl_repo")

import numpy as np
import ml_dtypes

import concourse.bass as bass
import concourse.mybir as mybir
import concourse.tile as tile
from concourse import bacc
from concourse.bass_utils import run_bass_kernel_spmd

F32 = mybir.dt.float32
F16 = mybir.dt.float16
BF16 = mybir.dt.bfloat16

CORES = 8
N_FULL = 4096
D_FULL = 256
T_BASE = 0.2
T_HATS = (1.0, 2.0, 4.0)
POISON = 1.0e6  # added to dist^2 of neg-diagonal entries (-> exp underflows to 0)

# engine choice per temperature index for the exp passes (tunable)
D_DTYPE = F16


def build(cores=CORES, N=N_FULL, D=D_FULL):
    """Builds the SPMD Bass kernel. Same NEFF runs on all cores."""
    M = N // cores            # local rows per core
    NEGCH = N // 128          # neg column chunks
    NCH = 2 * NEGCH           # total column chunks (neg then pos)
    KCH = D // 128            # contraction chunks for the distance matmul
    WIN = 128 // cores        # poison window width per neg chunk
    ISUB = (M + 127) // 128   # 128-row output subchunks
    NT = len(T_HATS)
    assert M % 128 == 0 and D % 128 == 0 and N % 128 == 0 and M <= 512
    assert WIN * NEGCH == M

    nc = bacc.Bacc(
        "TRN2",
        target_bir_lowering=False,
        debug=False,
        enable_asserts=True,
        num_devices=cores,
    )

    # ---- kernel I/O ----
    xT2_d = nc.dram_tensor("xT2", [D, M], BF16, kind="ExternalInput")
    yTn_d = nc.dram_tensor("yTn", [D, N], BF16, kind="ExternalInput")
    yTp_d = nc.dram_tensor("yTp", [D, N], BF16, kind="ExternalInput")
    yan_d = nc.dram_tensor("yan", [N, 258], BF16, kind="ExternalInput")
    yap_d = nc.dram_tensor("yap", [N, 258], BF16, kind="ExternalInput")
    sqyn_d = nc.dram_tensor("sqyn", [128, NEGCH], F32, kind="ExternalInput")
    sqyp_d = nc.dram_tensor("sqyp", [128, NEGCH], F32, kind="ExternalInput")
    sqxb_d = nc.dram_tensor("sqxb", [128, M], F32, kind="ExternalInput")
    poison_d = nc.dram_tensor("poison", [128, WIN], F32, kind="ExternalInput")
    ones_d = nc.dram_tensor("ones128", [128, 128], F32, kind="ExternalInput")
    loss_d = nc.dram_tensor("losspart", [128, 1], F32, kind="ExternalOutput")

    rg = [list(range(cores))]

    with tile.TileContext(nc) as tc:
        with (
            tc.tile_pool(name="consts", bufs=1) as consts,
            tc.tile_pool(name="dstore", bufs=1) as dstore,
            tc.tile_pool(name="stats", bufs=1) as stats,
            tc.tile_pool(name="dram", bufs=1, space="DRAM") as dram,
        ):
            # ---- load constants ----
            xT2 = consts.tile([128, KCH, M], BF16, name="xT2_sb")
            nc.sync.dma_start(xT2[:], xT2_d[:].rearrange("(k p) f -> p k f", p=128))
            yT = []
            for h, src in enumerate((yTn_d, yTp_d)):
                t = consts.tile([128, KCH, N], BF16, name=f"yT_sb{h}")
                nc.sync.dma_start(t[:], src[:].rearrange("(k p) f -> p k f", p=128))
                yT.append(t)
            ya = []
            for h, src in enumerate((yan_d, yap_d)):
                t = consts.tile([128, NEGCH, 258], BF16, name=f"ya_sb{h}")
                nc.sync.dma_start(t[:], src[:].rearrange("(c p) f -> p c f", p=128))
                ya.append(t)
            sqy = []
            for h, src in enumerate((sqyn_d, sqyp_d)):
                t = consts.tile([128, NEGCH], F32, name=f"sqy_sb{h}")
                nc.sync.dma_start(t[:], src[:])
                sqy.append(t)
            sqxb = consts.tile([128, M], F32, name="sqxb_sb")
            nc.sync.dma_start(sqxb[:], sqxb_d[:])
            poisonT = consts.tile([128, WIN], F32, name="poison_sb")
            nc.sync.dma_start(poisonT[:], poison_d[:])
            ones128 = consts.tile([128, 128], F32, name="ones_sb")
            nc.sync.dma_start(ones128[:], ones_d[:])

            # ---- persistent state ----
            d_sb = dstore.tile([128, NCH, M], D_DTYPE, name="d_sb")
            dsum = stats.tile([128, NEGCH], F32, name="dsum")
            scales = stats.tile([128, NT], F32, name="scales")
            colp = [stats.tile([128, NCH], F32, name=f"colp{t}") for t in range(NT)]
            colg = [stats.tile([128, NCH], F32, name=f"colg{t}") for t in range(NT)]
            lnic = [stats.tile([128, NCH], F32, name=f"lnic{t}") for t in range(NT)]
            sqc = [stats.tile([128, NCH], F32, name=f"sqc{t}") for t in range(NT)]
            V_sb = stats.tile([128, ISUB, D], F32, name="V_sb")
            lp = stats.tile([128, ISUB], F32, name="lp")
            msum = stats.tile([128, 1], F32, name="msum")
            sc_vec = stats.tile([128, NT], F32, name="sc_vec")
            inv_s = stats.tile([1, 1], F32, name="inv_s")
            s_sc = stats.tile([1, 1], F32, name="s_sc")

            # DRAM bounce buffers for collectives
            mean_in = dram.tile([128, 1], F32, name="mean_in")
            mean_out = dram.tile([128, 1], F32, name="mean_out", addr_space="Shared")
            col_in = [
                dram.tile([128, NCH], F32, name=f"col_in{t}") for t in range(NT)
            ]
            col_out = [
                dram.tile([128, NCH], F32, name=f"col_out{t}", addr_space="Shared")
                for t in range(NT)
            ]

            # ================= phase A: distances =================
            with (
                tc.tile_pool(name="pa", bufs=3, space="PSUM") as pa,
                tc.tile_pool(name="tmpp", bufs=4) as tmpp,
            ):
                def do_chunk(c):
                    pos = c >= NEGCH
                    cl = c - NEGCH if pos else c
                    ps = pa.tile([128, M], F32, name="ps_d")
                    for k in range(KCH):
                        nc.tensor.matmul(
                            ps[:],
                            yT[1 if pos else 0][:, k, cl * 128 : (cl + 1) * 128],
                            xT2[:, k, :],
                            start=(k == 0),
                            stop=(k == KCH - 1),
                        )
                    tmp = tmpp.tile([128, M], F32, name="tmp_d")
                    nc.vector.tensor_tensor(
                        tmp[:], ps[:], sqxb[:], mybir.AluOpType.add
                    )
                    if not pos:
                        nc.vector.tensor_tensor(
                            tmp[:, cl * WIN : (cl + 1) * WIN],
                            tmp[:, cl * WIN : (cl + 1) * WIN],
                            poisonT[:],
                            mybir.AluOpType.add,
                        )
                    nc.scalar.activation(
                        d_sb[:, c, :],
                        tmp[:],
                        mybir.ActivationFunctionType.Sqrt,
                        bias=sqy[1 if pos else 0][:, cl : cl + 1],
                        scale=1.0,
                        accum_out=dsum[:, cl : cl + 1] if pos else None,
                    )

                # pos chunks first: they feed the mean all-reduce
                for c in range(NEGCH, NCH):
                    do_chunk(c)

                # mean all-reduce (overlaps with the neg-chunk work below)
                dtot = stats.tile([128, 1], F32, name="dtot")
                nc.vector.reduce_sum(dtot[:], dsum[:], axis=mybir.AxisListType.X)
                nc.sync.dma_start(mean_in[:], dtot[:])
                nc.gpsimd.collective_compute(
                    "AllReduce",
                    mybir.AluOpType.add,
                    replica_groups=rg,
                    ins=[mean_in[:].opt()],
                    outs=[mean_out[:].opt()],
                )
                nc.sync.dma_start(msum[:], mean_out[:])

                for c in range(0, NEGCH):
                    do_chunk(c)

                # ---- scales from the mean ----
                with tc.tile_pool(name="psmall", bufs=1, space="PSUM") as psmall:
                    ps1 = psmall.tile([1, 1], F32, name="ps1")
                    nc.tensor.matmul(
                        ps1[:], msum[:], ones128[:, 0:1], start=True, stop=True
                    )
                    nc.scalar.copy(s_sc[:], ps1[:])
                    nc.vector.reciprocal(inv_s[:], s_sc[:])
                    nc.vector.memset(sc_vec[:], 0.0)
                    for t, th in enumerate(T_HATS):
                        coef = -th * (N * N) / T_BASE
                        nc.vector.tensor_scalar_mul(
                            sc_vec[0:1, t : t + 1], inv_s[0:1, 0:1], coef
                        )
                    psb = psmall.tile([128, NT], F32, name="psb")
                    nc.tensor.matmul(psb[:], ones128[:], sc_vec[0:128, :],
                                     start=True, stop=True)
                    nc.scalar.copy(scales[:], psb[:])

            # ================= phase B: column sums =================
            with tc.tile_pool(name="epool", bufs=3) as epool:
                for t in range(NT):
                    for c in range(NCH):
                        e = epool.tile([128, M], BF16, name="e_scr")
                        nc.scalar.activation(
                            e[:],
                            d_sb[:, c, :],
                            mybir.ActivationFunctionType.Exp,
                            bias=0.0,
                            scale=scales[:, t : t + 1],
                            accum_out=colp[t][:, c : c + 1],
                        )
                    nc.sync.dma_start(col_in[t][:], colp[t][:])
                    nc.gpsimd.collective_compute(
                        "AllReduce",
                        mybir.AluOpType.add,
                        replica_groups=rg,
                        ins=[col_in[t][:].opt()],
                        outs=[col_out[t][:].opt()],
                    )
                    nc.sync.dma_start(colg[t][:], col_out[t][:])

            # ================= phase C: normalize + output matmuls ======
            with (
                tc.tile_pool(name="kpool", bufs=4) as kpool,
                tc.tile_pool(name="pc", bufs=1, space="PSUM") as pc,
                tc.tile_pool(name="drain", bufs=4) as drain,
            ):
                for t in range(NT):
                    # ic = 1/sqrt(c);  ln(ic);  sqrt(c)
                    rc = stats.tile([128, NCH], F32, name=f"rc{t}")
                    nc.vector.reciprocal(rc[:], colg[t][:])
                    ict = stats.tile([128, NCH], F32, name=f"ict{t}")
                    nc.scalar.activation(
                        ict[:], rc[:], mybir.ActivationFunctionType.Sqrt
                    )
                    nc.scalar.activation(
                        lnic[t][:], ict[:], mybir.ActivationFunctionType.Ln
                    )
                    nc.scalar.activation(
                        sqc[t][:], colg[t][:], mybir.ActivationFunctionType.Sqrt
                    )
                    # write sqrt(c) into the aug column of y (col 257)
                    nc.vector.tensor_copy(ya[0][:, :, 257], sqc[t][:, 0:NEGCH])
                    nc.vector.tensor_copy(ya[1][:, :, 257], sqc[t][:, NEGCH:NCH])

                    psums = [
                        [
                            pc.tile([128, 258], F32, name=f"pch{h}_{i}",
                                    tag=f"pch{h}_{i}")
                            for i in range(ISUB)
                        ]
                        for h in range(2)
                    ]
                    for c in range(NCH):
                        pos = c >= NEGCH
                        cl = c - NEGCH if pos else c
                        kp = kpool.tile([128, M], BF16, name="kp_scr")
                        nc.scalar.activation(
                            kp[:],
                            d_sb[:, c, :],
                            mybir.ActivationFunctionType.Exp,
                            bias=lnic[t][:, c : c + 1],
                            scale=scales[:, t : t + 1],
                        )
                        for i in range(ISUB):
                            nc.tensor.matmul(
                                psums[1 if pos else 0][i][:],
                                kp[:, i * 128 : (i + 1) * 128],
                                ya[1 if pos else 0][:, cl, :],
                                start=(cl == 0),
                                stop=(cl == NEGCH - 1),
                            )

                    for i in range(ISUB):
                        pn, pp = psums[0][i], psums[1][i]
                        rn_s = drain.tile([128, 2], F32, name="rn_s")
                        rp_s = drain.tile([128, 2], F32, name="rp_s")
                        nc.vector.tensor_copy(rn_s[:], pn[:, 256:258])
                        nc.vector.tensor_copy(rp_s[:], pp[:, 256:258])
                        st = drain.tile([128, 1], F32, name="st")
                        nc.vector.tensor_tensor(
                            st[:], rn_s[:, 1:2], rp_s[:, 1:2], mybir.AluOpType.add
                        )
                        rinv = drain.tile([128, 1], F32, name="rinv")
                        nc.vector.reciprocal(rinv[:], st[:])
                        af = drain.tile([128, 1], F32, name="af")
                        bf = drain.tile([128, 1], F32, name="bf")
                        nc.vector.tensor_tensor(
                            af[:], rn_s[:, 0:1], rinv[:], mybir.AluOpType.mult
                        )
                        nc.vector.tensor_tensor(
                            bf[:], rp_s[:, 0:1], rinv[:], mybir.AluOpType.mult
                        )
                        u1 = drain.tile([128, D], F32, name="u1")
                        u2 = drain.tile([128, D], F32, name="u2")
                        nc.vector.tensor_scalar_mul(u1[:], pp[:, 0:D], af[:])
                        nc.vector.tensor_scalar_mul(u2[:], pn[:, 0:D], bf[:])
                        if t == 0:
                            nc.vector.tensor_tensor(
                                V_sb[:, i, :], u1[:], u2[:],
                                mybir.AluOpType.subtract,
                            )
                        else:
                            nc.vector.tensor_tensor(
                                V_sb[:, i, :], V_sb[:, i, :], u1[:],
                                mybir.AluOpType.add,
                            )
                            nc.vector.tensor_tensor(
                                V_sb[:, i, :], V_sb[:, i, :], u2[:],
                                mybir.AluOpType.subtract,
                            )

                # ---- loss partials ----
                for i in range(ISUB):
                    scr = drain.tile([128, D], F32, name="sq_scr")
                    nc.scalar.activation(
                        scr[:],
                        V_sb[:, i, :],
                        mybir.ActivationFunctionType.Square,
                        accum_out=lp[:, i : i + 1],
                    )
                lout = stats.tile([128, 1], F32, name="lout")
                nc.vector.reduce_sum(lout[:], lp[:], axis=mybir.AxisListType.X)
                nc.sync.dma_start(loss_d[:], lout[:])

    nc.compile()
    return nc


def prepare_inputs(x, y_pos, y_neg, cores=CORES):
    """Host-side input prep: shard, transpose, cast, norms, masks."""
    x = np.asarray(x, dtype=np.float32)
    y_pos = np.asarray(y_pos, dtype=np.float32)
    y_neg = np.asarray(y_neg, dtype=np.float32)
    N, D = x.shape
    M = N // cores
    NEGCH = N // 128
    WIN = 128 // cores
    bf = ml_dtypes.bfloat16

    def aug(y):
        a = np.zeros((N, 258), dtype=bf)
        a[:, :D] = y.astype(bf)
        a[:, 256] = bf(1.0)
        return a

    def sqmat(y):
        s = (y * y).sum(axis=1).astype(np.float32)  # [N]
        return np.ascontiguousarray(s.reshape(NEGCH, 128).T)  # [128, NEGCH]

    shared = {
        "yTn": np.ascontiguousarray(y_neg.T).astype(bf),
        "yTp": np.ascontiguousarray(y_pos.T).astype(bf),
        "yan": aug(y_neg),
        "yap": aug(y_pos),
        "sqyn": sqmat(y_neg),
        "sqyp": sqmat(y_pos),
        "ones128": np.ones((128, 128), dtype=np.float32),
    }
    in_maps = []
    for c in range(cores):
        xs = x[c::cores]  # [M, D]
        sqx = (xs * xs).sum(axis=1).astype(np.float32)  # [M]
        poison = np.zeros((128, WIN), dtype=np.float32)
        for q in range(WIN):
            poison[c + cores * q, q] = POISON
        m = dict(shared)
        m["xT2"] = np.ascontiguousarray((-2.0 * xs).T).astype(bf)
        m["sqxb"] = np.ascontiguousarray(
            np.broadcast_to(sqx[None, :], (128, M))
        )
        m["poison"] = poison
        in_maps.append(m)
    return in_maps


_CACHED = {}


def _get_nc(cores, N, D):
    key = (cores, N, D)
    if key not in _CACHED:
        _CACHED[key] = build(cores, N, D)
    return _CACHED[key]


def kernel(x, y_pos, y_neg, _trace=False, _tracekw=None):
    x = np.asarray(x)
    N, D = x.shape
    nc = _get_nc(CORES, N, D)
    in_maps = prepare_inputs(x, y_pos, y_neg, CORES)
    kw = dict(_tracekw or {})
    res = run_bass_kernel_spmd(
        nc, in_maps, core_ids=list(range(CORES)), trace=_trace, **kw
    )
    total = sum(float(res.results[c]["losspart"].sum()) for c in range(CORES))
    loss = np.float32(total / (N * D))
    out = np.array(loss, dtype=np.float32)
    if _trace:
        return out, res
    return out


if __name__ == "__main__":
    rng = np.random.default_rng(0)
    N, D = N_FULL, D_FULL
    x = rng.standard_normal((N, D)).astype(np.float32)
    yp = rng.standard_normal((N, D)).astype(np.float32)
    yn = rng.standard_normal((N, D)).astype(np.float32)
    print("loss:", kernel(x, yp, yn))
